# revision 32
# baseline (speedup 1.0000x reference)
"""PointPillarsScatter Trainium2 kernel (fp16 pipeline).

Reference op:
  canvas[b*NY*NX + y*NX + x] = voxel_features[p]        (scatter-set, 64 ch)
  out[:, :64]  = canvas -> [B, 64, NY, NX]
  out[:, 64:]  = transpose(map_fm, (0, 3, 2, 1))        (16 ch)

Strategy (8 NeuronCores, SPMD, data-parallel per sharding hint):
  core = batch*2 + y_half  (4 batches x 2 halves of NY=496 -> NYH=248 rows).

  Everything on-device runs in fp16 (correctness gate is rel_err < 2e-2;
  fp16 gives ~5e-4), which halves HBM traffic vs fp32 and lets the PE run
  at 1 column/cycle.  The scatter is a one-hot matmul on the TensorEngine:
    psum[128ch', 512cells] = feat[128slots, 128ch'].T @ S[128slots, 512]
  where S[s, n] = (pos[s] == n) is built with iota + is_equal (all-fp16 ->
  2x DVE mode; a quarter of them on the otherwise-idle GpSimd), and ch'
  packs the 64 channels of TWO 512-cell tiles (tile j -> psum partitions
  0:64, tile j+105 -> 64:128).  This fuses zero-fill + scatter + transpose
  into one PE op per 1024 cells.  With fp16 there is no hi/lo split: all
  128 slots hold points (CAP=128 per column).

  The canvas DRAM layout is [128, ACELL] fp16 (partition = half*64 + ch),
  so every store is one full-128-partition DMA with 8KB runs; the host
  reassembles the [64, NCELL] canvas from the two halves.

  map_fm is transposed with PE transpose (fp16 identity) in [<=128, 128]
  blocks directly into fp16 PSUM, copied 2 y-blocks at a time, and stored
  4 y-blocks per DMA (3456B contiguous runs).

  Pipeline shape: stores start after the first 8-pair group (~7us);
  feat loads are staggered on the sync queue between stores; map input
  loads ride the idle GpSimd (SWDGE) queue; map transpose work is
  front-loaded so the map output finishes before the scatter does.

Host side only computes index tables + shards/casts inputs; all FP math
(scatter + transpose) runs on device.
"""

import sys

for _p in ("/opt/trn_rl_repo",):
    if _p not in sys.path:
        sys.path.insert(0, _p)

import numpy as np

# problem constants (hardcoded per contract)
B, NPTS, C, NY, NX, CM = 4, 48000, 64, 496, 432, 16
NYH = NY // 2            # 248 rows per core
NCORE = 8
NCELL = NYH * NX         # 107136 cells per core
TILE = 512               # cells per channel-block
NT = (NCELL + TILE - 1) // TILE          # 210 tiles (last has 128 cells)
NP = (NT + 1) // 2                       # 105 pairs: tile j with tile j+NP
ACELL = NP * TILE                        # 53760 cells in the A half
BCELL = NCELL - ACELL                    # 53376 cells in the B half
CAP = 128                # point slots per column (fp16: no hi/lo split)
SG = 8                   # pairs per canvas-store group (105 = 13*8 + 1)
NSG = (NP + SG - 1) // SG                # 14 store groups
FBMAX = 12               # max feat columns per group load
FBLOOK = 4               # feat groups prefetched ahead
YB = 8                   # map y rows per transpose block ( YB*CM = 128 )
NYB = NYH // YB          # 31 y-blocks
NMU = (NYB + 1) // 2     # 16 map units (2 y-blocks each; last has 1)
MBG = 4                  # map y-blocks per store DMA (8 stores)
MAP_AT = 1               # scatter group at which map work starts
MAP_END = 13             # scatter group by which all map work is emitted
NWARM = 10               # dummy matmuls to warm the PE HAM clock gate
XCH = [(0, 128), (128, 128), (256, 128), (384, 48)]   # x chunks of NX=432

_prog_cache = {}


def _build_program(ncols, chunks, hmax):
    """Build the SPMD Bass program (identical for all 8 cores)."""
    from concourse import bacc, mybir, tile

    f16 = mybir.dt.float16
    f32 = mybir.dt.float32

    nc = bacc.Bacc(trn_type="TRN2", target_bir_lowering=False)

    # slot-major layout: partition s reads one contiguous run per load
    feat_d = nc.dram_tensor("feat", [hmax, ncols * 2 * C], f16,
                            kind="ExternalInput")
    post_d = nc.dram_tensor("post", [hmax, ncols], f32, kind="ExternalInput")
    map_d = nc.dram_tensor("mapin", [NX, NYH * CM], f16, kind="ExternalInput")
    canv_d = nc.dram_tensor("canv", [128, ACELL], f16, kind="ExternalOutput")
    mapo_d = nc.dram_tensor("mapo", [128, NYB * NX], f16,
                            kind="ExternalOutput")

    colbase = np.concatenate([[0], np.cumsum(chunks)]).astype(np.int64)

    with tile.TileContext(nc) as tc:
        with (
            tc.tile_pool(name="const", bufs=1) as cpool,
            tc.tile_pool(name="fpool", bufs=FBLOOK + 1) as fpool,
            tc.tile_pool(name="spool", bufs=6) as spool,
            tc.tile_pool(name="stg", bufs=3) as stpool,
            tc.tile_pool(name="mstg", bufs=2) as mstpool,
            tc.tile_pool(name="pscat", bufs=3, space="PSUM") as pspool,
            tc.tile_pool(name="pwarm", bufs=1, space="PSUM") as pwpool,
        ):
            # PE warmup: ~4us of back-to-back dummy matmuls during the load
            # phase so the HAM clock gate reaches 2.4GHz before real work
            # (without this, every matmul runs at the cold 1.2GHz rate)
            wsrc = cpool.tile([128, TILE], f16)
            nc.gpsimd.memset(wsrc[:], 0.0)
            wps = pwpool.tile([128, TILE], f32, name="wps")
            for _ in range(NWARM):
                nc.tensor.matmul(out=wps[:], lhsT=wsrc[:, 0:128],
                                 rhs=wsrc[:], start=True, stop=True)

            # scatter-critical load first: pos table (alone on its queue)
            posT = cpool.tile([hmax, ncols], f32)
            nc.scalar.dma_start(out=posT[:], in_=post_d[:])

            fbs = {}

            def load_fb(g):
                p0, p1 = g * SG, min((g + 1) * SG, NP)
                c0, c1 = int(colbase[p0]), int(colbase[p1])
                assert c1 - c0 <= FBMAX, (c0, c1)
                fb = fpool.tile([hmax, FBMAX * 2 * C], f16, name="fb")
                nc.sync.dma_start(out=fb[:, :(c1 - c0) * 2 * C],
                                  in_=feat_d[:, c0 * 2 * C:c1 * 2 * C])
                fbs[g] = (fb, c0, c1)

            for g in range(min(FBLOOK, NSG)):
                load_fb(g)

            # constants built on-device (no DMA -> no queue contention)
            iota_i = cpool.tile([128, TILE], mybir.dt.int32)
            nc.gpsimd.iota(iota_i[:], pattern=[[1, TILE]], base=0,
                           channel_multiplier=0)
            iota_f32 = cpool.tile([128, TILE], f32)
            nc.gpsimd.tensor_copy(iota_f32[:], iota_i[:])
            iota_f = cpool.tile([128, TILE], f16)
            nc.vector.tensor_copy(iota_f[:], iota_f32[:])

            # ---- map path: pure DMA (X-bar transpose loads + stores) ----
            # The map never touches PE/PSUM/DVE: each y-block [432, 128]
            # region of map_d is transpose-loaded into SBUF as [128, 432],
            # then MBG blocks are stored with one DMA.  This keeps the PE
            # stream dense (scatter matmuls only) so the HAM clock gate
            # stays warm.
            mstate = {"ms": None}

            def emit_map_block(kb):
                if kb % MBG == 0:
                    mstate["ms"] = mstpool.tile([128, MBG * NX], f16,
                                                name="ms")
                ms = mstate["ms"]
                off = (kb % MBG) * NX
                nc.sync.dma_start(out=ms[:, off:off + NX],
                                  in_=map_d[:, kb * 128:(kb + 1) * 128],
                                  transpose=True)
                if kb % MBG == MBG - 1 or kb == NYB - 1:
                    blk0 = kb - kb % MBG
                    wm = (min(blk0 + MBG, NYB) - blk0) * NX
                    nc.sync.dma_start(
                        out=mapo_d[:, blk0 * NX:blk0 * NX + wm],
                        in_=ms[:, :wm])

            # ---- scatter main loop ----
            emitted_units = 0
            ndp = 0     # global double-pair counter (for copy-engine split)
            for g in range(NSG):
                p0, p1 = g * SG, min((g + 1) * SG, NP)
                fb, c0, c1 = fbs[g]
                stg = stpool.tile([128, SG * TILE], f16, name="stg")
                pr = p0
                while pr < p1:
                    npair = min(2, p1 - pr)
                    ps = pspool.tile([128, 2 * TILE], f32, name="ps")
                    for q in range(npair):
                        pcur = pr + q
                        nck = int(chunks[pcur])
                        for k in range(nck):
                            col = int(colbase[pcur]) + k
                            s_t = spool.tile([hmax, TILE], f16, name="s_t")
                            nc.vector.tensor_scalar(
                                out=s_t[:], in0=iota_f[:hmax],
                                scalar1=posT[:, col:col + 1], scalar2=None,
                                op0=mybir.AluOpType.is_equal)
                            nc.tensor.matmul(
                                out=ps[:, q * TILE:(q + 1) * TILE],
                                lhsT=fb[:, (col - c0) * 2 * C:
                                        (col - c0 + 1) * 2 * C],
                                rhs=s_t[:],
                                start=(k == 0), stop=(k == nck - 1))
                    off = (pr - p0) * TILE
                    # most copies on ACT; every 5th on DVE for balance
                    if ndp % 5 == 4:
                        nc.vector.tensor_copy(
                            out=stg[:, off:off + npair * TILE],
                            in_=ps[:, :npair * TILE])
                    else:
                        nc.scalar.copy(
                            out=stg[:, off:off + npair * TILE],
                            in_=ps[:, :npair * TILE])
                    ndp += 1
                    pr += npair
                a0 = p0 * TILE
                wa = (p1 - p0) * TILE
                nc.sync.dma_start(out=canv_d[:, a0:a0 + wa],
                                  in_=stg[:, :wa])
                if g + FBLOOK < NSG:
                    load_fb(g + FBLOOK)
                if g >= MAP_AT:
                    while (emitted_units < NYB
                           and emitted_units * (MAP_END - MAP_AT)
                           < (g + 1 - MAP_AT) * NYB):
                        emit_map_block(emitted_units)
                        emitted_units += 1
            while emitted_units < NYB:
                emit_map_block(emitted_units)
                emitted_units += 1

    nc.finalize()
    return nc


def _host_prep(voxel_features, coords, map_fm):
    """Shard points by core, build fp16 feature/pos tables (index work)."""
    vf = np.asarray(voxel_features)
    cd = np.asarray(coords)
    mf = np.asarray(map_fm)
    if mf.ndim == 5:
        mf = np.squeeze(mf, 3)

    b = cd[:, 0].astype(np.int64)
    y = cd[:, 2].astype(np.int64)
    x = cd[:, 3].astype(np.int64)
    valid = (b >= 0) & (b < B) & (y >= 0) & (y < NY) & (x >= 0) & (x < NX)
    b, y, x = b[valid], y[valid], x[valid]
    vfv = np.ascontiguousarray(vf[valid]).astype(np.float16)

    half = (y >= NYH).astype(np.int64)
    core = b * 2 + half
    lcell = (y - half * NYH) * NX + x
    t = lcell // TILE          # 512-cell tile id
    pos = lcell - t * TILE     # position within tile (= matmul column)
    pair = t % NP              # tile j pairs with tile j+NP
    blk = t // NP              # channel block within the pair

    key = core * NP + pair
    order = np.argsort(key, kind="stable")
    ks = key[order]
    counts = np.bincount(ks, minlength=NCORE * NP)
    kmax = counts.reshape(NCORE, NP).max(axis=0)
    # table height: round max points-per-column up to a multiple of 16
    hmax = int(min(CAP, max(16, -(-int(kmax.max()) // 16) * 16)))
    chunks = np.maximum((kmax + hmax - 1) // hmax, 1)
    for g in range(0, NP, SG):
        need = int(chunks[g:g + SG].sum())
        if need > FBMAX:
            raise ValueError("pair group needs %d cols > FBMAX=%d"
                             % (need, FBMAX))
    ncols = int(chunks.sum())
    colbase = np.concatenate([[0], np.cumsum(chunks)]).astype(np.int64)

    starts = np.concatenate([[0], np.cumsum(counts)]).astype(np.int64)
    rank = np.arange(len(ks), dtype=np.int64) - starts[ks]

    co = core[order]
    po = pair[order]
    bo = blk[order]
    colo = colbase[po] + rank // hmax
    slot = rank % hmax

    feat = np.zeros((NCORE, hmax, ncols, 2 * C), np.float16)
    post = np.full((NCORE, hmax, ncols), -1.0, np.float32)
    ccol = bo[:, None] * C + np.arange(C)[None, :]
    feat[co[:, None], slot[:, None], colo[:, None], ccol] = vfv[order]
    post[co, slot, colo] = pos[order].astype(np.float32)

    maps = []
    for core_id in range(NCORE):
        bb, hh = core_id // 2, core_id % 2
        maps.append(np.ascontiguousarray(
            mf[bb, :, hh * NYH:(hh + 1) * NYH, :]).astype(
                np.float16).reshape(NX, NYH * CM))
    return feat, post, maps, ncols, chunks, hmax


def kernel(voxel_features, coords, batch_size=None, map_fm=None,
           trace=False, _return_results=False):
    from concourse.bass_utils import run_bass_kernel_spmd

    feat, post, maps, ncols, chunks, hmax = _host_prep(
        voxel_features, coords, map_fm)

    ckey = (ncols, hmax, tuple(int(c) for c in chunks))
    if ckey not in _prog_cache:
        _prog_cache.clear()
        _prog_cache[ckey] = _build_program(ncols, chunks, hmax)
    nc = _prog_cache[ckey]

    in_maps = [
        {"feat": feat[i].reshape(hmax, -1), "post": post[i],
         "mapin": maps[i]}
        for i in range(NCORE)
    ]
    res = run_bass_kernel_spmd(nc, in_maps, list(range(NCORE)), trace=trace)

    out = np.empty((B, C + CM, NY, NX), np.float32)
    for core_id in range(NCORE):
        bb, hh = core_id // 2, core_id % 2
        canv = res.results[core_id]["canv"]          # [128, ACELL] f16
        full = np.concatenate(
            [canv[0:C], canv[C:, :BCELL]], axis=1).astype(np.float32)
        out[bb, :C, hh * NYH:(hh + 1) * NYH, :] = full.reshape(C, NYH, NX)
        mo = res.results[core_id]["mapo"]            # [128, NYB*NX] f16
        out[bb, C:, hh * NYH:(hh + 1) * NYH, :] = (
            mo.reshape(YB, CM, NYB, NX).transpose(1, 2, 0, 3)
            .astype(np.float32).reshape(CM, NYH, NX))
    if _return_results:
        return out, res
    return out


# revision 41
# speedup vs baseline: 1.6718x; 1.6718x over previous
"""PointPillarsScatter Trainium2 kernel (fp16 pipeline).

Reference op:
  canvas[b*NY*NX + y*NX + x] = voxel_features[p]        (scatter-set, 64 ch)
  out[:, :64]  = canvas -> [B, 64, NY, NX]
  out[:, 64:]  = transpose(map_fm, (0, 3, 2, 1))        (16 ch)

Strategy (8 NeuronCores, SPMD, data-parallel per sharding hint):
  core = batch*2 + y_half  (4 batches x 2 halves of NY=496 -> NYH=248 rows).

  Everything on-device runs in fp16 (correctness gate is rel_err < 2e-2;
  fp16 gives ~5e-4), which halves HBM traffic vs fp32 and lets the PE run
  at 1 column/cycle.  The scatter is a one-hot matmul on the TensorEngine:
    psum[128ch', 512cells] = feat[128slots, 128ch'].T @ S[128slots, 512]
  where S[s, n] = (pos[s] == n) is built with iota + is_equal (all-fp16 ->
  2x DVE mode; a quarter of them on the otherwise-idle GpSimd), and ch'
  packs the 64 channels of TWO 512-cell tiles (tile j -> psum partitions
  0:64, tile j+105 -> 64:128).  This fuses zero-fill + scatter + transpose
  into one PE op per 1024 cells.  With fp16 there is no hi/lo split: all
  128 slots hold points (CAP=128 per column).

  The canvas DRAM layout is [128, ACELL] fp16 (partition = half*64 + ch),
  so every store is one full-128-partition DMA with 8KB runs; the host
  reassembles the [64, NCELL] canvas from the two halves.

  map_fm is transposed with PE transpose (fp16 identity) in [<=128, 128]
  blocks directly into fp16 PSUM, copied 2 y-blocks at a time, and stored
  4 y-blocks per DMA (3456B contiguous runs).

  Pipeline shape: stores start after the first 8-pair group (~7us);
  feat loads are staggered on the sync queue between stores; map input
  loads ride the idle GpSimd (SWDGE) queue; map transpose work is
  front-loaded so the map output finishes before the scatter does.

Host side only computes index tables + shards/casts inputs; all FP math
(scatter + transpose) runs on device.
"""

import sys

for _p in ("/opt/trn_rl_repo",):
    if _p not in sys.path:
        sys.path.insert(0, _p)

import numpy as np

# problem constants (hardcoded per contract)
B, NPTS, C, NY, NX, CM = 4, 48000, 64, 496, 432, 16
NYH = NY // 2            # 248 rows per core
NCORE = 8
NCELL = NYH * NX         # 107136 cells per core
TILE = 512               # cells per channel-block
NT = (NCELL + TILE - 1) // TILE          # 210 tiles (last has 128 cells)
NP = (NT + 1) // 2                       # 105 pairs: tile j with tile j+NP
ACELL = NP * TILE                        # 53760 cells in the A half
BCELL = NCELL - ACELL                    # 53376 cells in the B half
CAP = 128                # point slots per column (fp16: no hi/lo split)
SG = 8                   # pairs per canvas-store group (105 = 13*8 + 1)
NSG = (NP + SG - 1) // SG                # 14 store groups
FBMAX = 12               # max feat columns per group load
FBLOOK = 4               # feat groups prefetched ahead
YB = 8                   # map y rows per transpose block ( YB*CM = 128 )
NYB = NYH // YB          # 31 y-blocks
NMU = (NYB + 1) // 2     # 16 map units (2 y-blocks each; last has 1)
MBG = 4                  # map y-blocks per store DMA (8 stores)
MAP_AT = 2               # scatter group at which map transposes start
MAP_END = 13             # scatter group by which all map work is emitted
NWARM = 10               # dummy matmuls to warm the PE HAM clock gate
REWARM = (2, 5, 8)       # groups after which a 12-matmul re-warm burst runs
MHALF = 16 * 128         # map columns in the first half-load (y-blocks 0-15)
XCH = [(0, 128), (128, 128), (256, 128), (384, 48)]   # x chunks of NX=432

_prog_cache = {}


def _build_program(ncols, chunks, hmax):
    """Build the SPMD Bass program (identical for all 8 cores)."""
    from concourse import bacc, mybir, tile
    from concourse.masks import make_identity

    f16 = mybir.dt.float16
    f32 = mybir.dt.float32

    nc = bacc.Bacc(trn_type="TRN2", target_bir_lowering=False)

    # slot-major layout: partition s reads one contiguous run per load
    feat_d = nc.dram_tensor("feat", [hmax, ncols * 2 * C], f16,
                            kind="ExternalInput")
    post_d = nc.dram_tensor("post", [hmax, ncols], f32, kind="ExternalInput")
    map_d = nc.dram_tensor("mapin", [NX, NYH * CM], f16, kind="ExternalInput")
    canv_d = nc.dram_tensor("canv", [128, ACELL], f16, kind="ExternalOutput")
    mapo_d = nc.dram_tensor("mapo", [128, NYB * NX], f16,
                            kind="ExternalOutput")

    colbase = np.concatenate([[0], np.cumsum(chunks)]).astype(np.int64)

    with tile.TileContext(nc) as tc:
        with (
            tc.tile_pool(name="const", bufs=1) as cpool,
            tc.tile_pool(name="fpool", bufs=FBLOOK + 1) as fpool,
            tc.tile_pool(name="spool", bufs=6) as spool,
            tc.tile_pool(name="stg", bufs=3) as stpool,
            tc.tile_pool(name="mstg", bufs=2) as mstpool,
            tc.tile_pool(name="mtin", bufs=1) as mtpool,
            tc.tile_pool(name="pscat", bufs=2, space="PSUM") as pspool,
            tc.tile_pool(name="pmap", bufs=2, space="PSUM") as pmpool,
            tc.tile_pool(name="pwarm", bufs=1, space="PSUM") as pwpool,
        ):
            # PE warmup: back-to-back dummy matmuls with no inputs so the
            # HAM clock gate reaches 2.4GHz before real work (without this,
            # matmuls run at the cold 1.2GHz rate).  Re-warm bursts are
            # emitted inside the main loop: one PE stall >3.4us re-throttles
            # the clock and the normal stream is never dense enough to
            # recover on its own.
            wsrc = cpool.tile([128, TILE], f16)
            nc.gpsimd.memset(wsrc[:], 0.0)

            def warm_burst(n):
                wps = pwpool.tile([128, TILE], f32, name="wps")
                for _ in range(n):
                    nc.tensor.matmul(out=wps[:], lhsT=wsrc[:, 0:128],
                                     rhs=wsrc[:], start=True, stop=True)

            warm_burst(NWARM)

            # scatter-critical load first: pos table (alone on its queue)
            posT = cpool.tile([hmax, ncols], f32)
            nc.scalar.dma_start(out=posT[:], in_=post_d[:])

            fbs = {}

            def load_fb(g):
                p0, p1 = g * SG, min((g + 1) * SG, NP)
                c0, c1 = int(colbase[p0]), int(colbase[p1])
                assert c1 - c0 <= FBMAX, (c0, c1)
                fb = fpool.tile([hmax, FBMAX * 2 * C], f16, name="fb")
                nc.sync.dma_start(out=fb[:, :(c1 - c0) * 2 * C],
                                  in_=feat_d[:, c0 * 2 * C:c1 * 2 * C])
                fbs[g] = (fb, c0, c1)

            for g in range(min(FBLOOK, NSG)):
                load_fb(g)

            # constants built on-device (no DMA -> no queue contention)
            iota_i = cpool.tile([128, TILE], mybir.dt.int32)
            nc.gpsimd.iota(iota_i[:], pattern=[[1, TILE]], base=0,
                           channel_multiplier=0)
            iota_f32 = cpool.tile([128, TILE], f32)
            nc.gpsimd.tensor_copy(iota_f32[:], iota_i[:])
            iota_f = cpool.tile([128, TILE], f16)
            nc.vector.tensor_copy(iota_f[:], iota_f32[:])
            ident = cpool.tile([128, 128], f16)
            make_identity(nc, ident[:])

            # map input rides the idle GpSimd (SWDGE) queue, split into
            # y-halves so the first map units unblock early
            mts = []
            for x0, w in XCH:
                mt = mtpool.tile([128, NYH * CM], f16, tag="mt%d" % x0)
                mts.append(mt)
            for xi, (x0, w) in enumerate(XCH):
                nc.gpsimd.dma_start(out=mts[xi][:w, :MHALF],
                                    in_=map_d[x0:x0 + w, :MHALF])
            for xi, (x0, w) in enumerate(XCH):
                nc.gpsimd.dma_start(out=mts[xi][:w, MHALF:],
                                    in_=map_d[x0:x0 + w, MHALF:])

            # ---- map transpose machinery ----
            # unit k2 covers y-blocks 2*k2, 2*k2+1 (last unit: 1 block)
            mstate = {"ms": None}

            def emit_map_unit(k2):
                nb = 2 if 2 * k2 + 1 < NYB else 1
                pm = pmpool.tile([128, 2 * NX], f16, name="pm")
                for j in range(nb):
                    kb = 2 * k2 + j
                    for xi, (x0, w) in enumerate(XCH):
                        nc.tensor.transpose(
                            out=pm[:, j * NX + x0:j * NX + x0 + w],
                            in_=mts[xi][:w, kb * 128:(kb + 1) * 128],
                            identity=ident[:w, :w])
                if k2 % 2 == 0:
                    mstate["ms"] = mstpool.tile([128, MBG * NX], f16,
                                                name="ms")
                ms = mstate["ms"]
                off = (k2 % 2) * 2 * NX
                nc.vector.tensor_copy(out=ms[:, off:off + nb * NX],
                                      in_=pm[:, :nb * NX])
                if k2 % 2 == 1 or k2 == NMU - 1:
                    blk0 = (k2 - k2 % 2) * 2
                    wm = (min(blk0 + MBG, NYB) - blk0) * NX
                    nc.sync.dma_start(
                        out=mapo_d[:, blk0 * NX:blk0 * NX + wm],
                        in_=ms[:, :wm])

            # ---- scatter main loop ----
            emitted_units = 0
            ndp = 0     # global double-pair counter (for copy-engine split)
            for g in range(NSG):
                p0, p1 = g * SG, min((g + 1) * SG, NP)
                fb, c0, c1 = fbs[g]
                stg = stpool.tile([128, SG * TILE], f16, name="stg")
                pr = p0
                while pr < p1:
                    npair = min(2, p1 - pr)
                    ps = pspool.tile([128, 2 * TILE], f32, name="ps")
                    for q in range(npair):
                        pcur = pr + q
                        nck = int(chunks[pcur])
                        for k in range(nck):
                            col = int(colbase[pcur]) + k
                            s_t = spool.tile([hmax, TILE], f16, name="s_t")
                            nc.vector.tensor_scalar(
                                out=s_t[:], in0=iota_f[:hmax],
                                scalar1=posT[:, col:col + 1], scalar2=None,
                                op0=mybir.AluOpType.is_equal)
                            nc.tensor.matmul(
                                out=ps[:, q * TILE:(q + 1) * TILE],
                                lhsT=fb[:, (col - c0) * 2 * C:
                                        (col - c0 + 1) * 2 * C],
                                rhs=s_t[:],
                                start=(k == 0), stop=(k == nck - 1))
                    off = (pr - p0) * TILE
                    # most copies on ACT; every 5th on DVE for balance
                    if ndp % 5 == 4:
                        nc.vector.tensor_copy(
                            out=stg[:, off:off + npair * TILE],
                            in_=ps[:, :npair * TILE])
                    else:
                        nc.scalar.copy(
                            out=stg[:, off:off + npair * TILE],
                            in_=ps[:, :npair * TILE])
                    ndp += 1
                    pr += npair
                a0 = p0 * TILE
                wa = (p1 - p0) * TILE
                nc.sync.dma_start(out=canv_d[:, a0:a0 + wa],
                                  in_=stg[:, :wa])
                if g + FBLOOK < NSG:
                    load_fb(g + FBLOOK)
                if g in REWARM:
                    warm_burst(12)
                if g >= MAP_AT:
                    while (emitted_units < NMU
                           and emitted_units * (MAP_END - MAP_AT)
                           < (g + 1 - MAP_AT) * NMU):
                        emit_map_unit(emitted_units)
                        emitted_units += 1
            while emitted_units < NMU:
                emit_map_unit(emitted_units)
                emitted_units += 1

    nc.finalize()
    return nc


def _host_prep(voxel_features, coords, map_fm):
    """Shard points by core, build fp16 feature/pos tables (index work)."""
    vf = np.asarray(voxel_features)
    cd = np.asarray(coords)
    mf = np.asarray(map_fm)
    if mf.ndim == 5:
        mf = np.squeeze(mf, 3)

    b = cd[:, 0].astype(np.int64)
    y = cd[:, 2].astype(np.int64)
    x = cd[:, 3].astype(np.int64)
    valid = (b >= 0) & (b < B) & (y >= 0) & (y < NY) & (x >= 0) & (x < NX)
    b, y, x = b[valid], y[valid], x[valid]
    vfv = np.ascontiguousarray(vf[valid]).astype(np.float16)

    half = (y >= NYH).astype(np.int64)
    core = b * 2 + half
    lcell = (y - half * NYH) * NX + x
    t = lcell // TILE          # 512-cell tile id
    pos = lcell - t * TILE     # position within tile (= matmul column)
    pair = t % NP              # tile j pairs with tile j+NP
    blk = t // NP              # channel block within the pair

    key = core * NP + pair
    order = np.argsort(key, kind="stable")
    ks = key[order]
    counts = np.bincount(ks, minlength=NCORE * NP)
    kmax = counts.reshape(NCORE, NP).max(axis=0)
    # table height: round max points-per-column up to a multiple of 16
    hmax = int(min(CAP, max(16, -(-int(kmax.max()) // 16) * 16)))
    chunks = np.maximum((kmax + hmax - 1) // hmax, 1)
    for g in range(0, NP, SG):
        need = int(chunks[g:g + SG].sum())
        if need > FBMAX:
            raise ValueError("pair group needs %d cols > FBMAX=%d"
                             % (need, FBMAX))
    ncols = int(chunks.sum())
    colbase = np.concatenate([[0], np.cumsum(chunks)]).astype(np.int64)

    starts = np.concatenate([[0], np.cumsum(counts)]).astype(np.int64)
    rank = np.arange(len(ks), dtype=np.int64) - starts[ks]

    co = core[order]
    po = pair[order]
    bo = blk[order]
    colo = colbase[po] + rank // hmax
    slot = rank % hmax

    feat = np.zeros((NCORE, hmax, ncols, 2 * C), np.float16)
    post = np.full((NCORE, hmax, ncols), -1.0, np.float32)
    ccol = bo[:, None] * C + np.arange(C)[None, :]
    feat[co[:, None], slot[:, None], colo[:, None], ccol] = vfv[order]
    post[co, slot, colo] = pos[order].astype(np.float32)

    maps = []
    for core_id in range(NCORE):
        bb, hh = core_id // 2, core_id % 2
        maps.append(np.ascontiguousarray(
            mf[bb, :, hh * NYH:(hh + 1) * NYH, :]).astype(
                np.float16).reshape(NX, NYH * CM))
    return feat, post, maps, ncols, chunks, hmax


def kernel(voxel_features, coords, batch_size=None, map_fm=None,
           trace=False, _return_results=False):
    from concourse.bass_utils import run_bass_kernel_spmd

    feat, post, maps, ncols, chunks, hmax = _host_prep(
        voxel_features, coords, map_fm)

    ckey = (ncols, hmax, tuple(int(c) for c in chunks))
    if ckey not in _prog_cache:
        _prog_cache.clear()
        _prog_cache[ckey] = _build_program(ncols, chunks, hmax)
    nc = _prog_cache[ckey]

    in_maps = [
        {"feat": feat[i].reshape(hmax, -1), "post": post[i],
         "mapin": maps[i]}
        for i in range(NCORE)
    ]
    res = run_bass_kernel_spmd(nc, in_maps, list(range(NCORE)), trace=trace)

    out = np.empty((B, C + CM, NY, NX), np.float32)
    for core_id in range(NCORE):
        bb, hh = core_id // 2, core_id % 2
        canv = res.results[core_id]["canv"]          # [128, ACELL] f16
        full = np.concatenate(
            [canv[0:C], canv[C:, :BCELL]], axis=1).astype(np.float32)
        out[bb, :C, hh * NYH:(hh + 1) * NYH, :] = full.reshape(C, NYH, NX)
        mo = res.results[core_id]["mapo"]            # [128, NYB*NX] f16
        out[bb, C:, hh * NYH:(hh + 1) * NYH, :] = (
            mo.reshape(YB, CM, NYB, NX).transpose(1, 2, 0, 3)
            .astype(np.float32).reshape(CM, NYH, NX))
    if _return_results:
        return out, res
    return out


# revision 43
# speedup vs baseline: 1.7082x; 1.0217x over previous
"""PointPillarsScatter Trainium2 kernel (fp16 pipeline).

Reference op:
  canvas[b*NY*NX + y*NX + x] = voxel_features[p]        (scatter-set, 64 ch)
  out[:, :64]  = canvas -> [B, 64, NY, NX]
  out[:, 64:]  = transpose(map_fm, (0, 3, 2, 1))        (16 ch)

Strategy (8 NeuronCores, SPMD, data-parallel per sharding hint):
  core = batch*2 + y_half  (4 batches x 2 halves of NY=496 -> NYH=248 rows).

  Everything on-device runs in fp16 (correctness gate is rel_err < 2e-2;
  fp16 gives ~5e-4), which halves HBM traffic vs fp32 and lets the PE run
  at 1 column/cycle.  The scatter is a one-hot matmul on the TensorEngine:
    psum[128ch', 512cells] = feat[128slots, 128ch'].T @ S[128slots, 512]
  where S[s, n] = (pos[s] == n) is built with iota + is_equal (all-fp16 ->
  2x DVE mode; a quarter of them on the otherwise-idle GpSimd), and ch'
  packs the 64 channels of TWO 512-cell tiles (tile j -> psum partitions
  0:64, tile j+105 -> 64:128).  This fuses zero-fill + scatter + transpose
  into one PE op per 1024 cells.  With fp16 there is no hi/lo split: all
  128 slots hold points (CAP=128 per column).

  The canvas DRAM layout is [128, ACELL] fp16 (partition = half*64 + ch),
  so every store is one full-128-partition DMA with 8KB runs; the host
  reassembles the [64, NCELL] canvas from the two halves.

  map_fm is transposed with PE transpose (fp16 identity) in [<=128, 128]
  blocks directly into fp16 PSUM, copied 2 y-blocks at a time, and stored
  4 y-blocks per DMA (3456B contiguous runs).

  Pipeline shape: stores start after the first 8-pair group (~7us);
  feat loads are staggered on the sync queue between stores; map input
  loads ride the idle GpSimd (SWDGE) queue; map transpose work is
  front-loaded so the map output finishes before the scatter does.

Host side only computes index tables + shards/casts inputs; all FP math
(scatter + transpose) runs on device.
"""

import sys

for _p in ("/opt/trn_rl_repo",):
    if _p not in sys.path:
        sys.path.insert(0, _p)

import numpy as np

# problem constants (hardcoded per contract)
B, NPTS, C, NY, NX, CM = 4, 48000, 64, 496, 432, 16
NYH = NY // 2            # 248 rows per core
NCORE = 8
NCELL = NYH * NX         # 107136 cells per core
TILE = 512               # cells per channel-block
NT = (NCELL + TILE - 1) // TILE          # 210 tiles (last has 128 cells)
NP = (NT + 1) // 2                       # 105 pairs: tile j with tile j+NP
ACELL = NP * TILE                        # 53760 cells in the A half
BCELL = NCELL - ACELL                    # 53376 cells in the B half
CAP = 128                # point slots per column (fp16: no hi/lo split)
SG = 8                   # pairs per canvas-store group (105 = 13*8 + 1)
NSG = (NP + SG - 1) // SG                # 14 store groups
FBMAX = 12               # max feat columns per group load
FBLOOK = 4               # feat groups prefetched ahead
YB = 8                   # map y rows per transpose block ( YB*CM = 128 )
NYB = NYH // YB          # 31 y-blocks
NMU = (NYB + 1) // 2     # 16 map units (2 y-blocks each; last has 1)
MBG = 4                  # map y-blocks per store DMA (8 stores)
MAP_AT = 2               # scatter group at which map transposes start
MAP_END = 13             # scatter group by which all map work is emitted
NWARM = 10               # dummy matmuls to warm the PE HAM clock gate
REWARM = ()              # groups after which a re-warm burst runs
MHALF = 16 * 128         # map columns in the first half-load (y-blocks 0-15)
XCH = [(0, 128), (128, 128), (256, 128), (384, 48)]   # x chunks of NX=432

_prog_cache = {}


def _build_program(ncols, chunks, hmax):
    """Build the SPMD Bass program (identical for all 8 cores)."""
    from concourse import bacc, mybir, tile
    from concourse.masks import make_identity

    f16 = mybir.dt.float16
    f32 = mybir.dt.float32

    nc = bacc.Bacc(trn_type="TRN2", target_bir_lowering=False)

    # slot-major layout: partition s reads one contiguous run per load
    feat_d = nc.dram_tensor("feat", [hmax, ncols * 2 * C], f16,
                            kind="ExternalInput")
    post_d = nc.dram_tensor("post", [hmax, ncols], f32, kind="ExternalInput")
    map_d = nc.dram_tensor("mapin", [NX, NYH * CM], f16, kind="ExternalInput")
    canv_d = nc.dram_tensor("canv", [128, ACELL], f16, kind="ExternalOutput")
    mapo_d = nc.dram_tensor("mapo", [128, NYB * NX], f16,
                            kind="ExternalOutput")

    colbase = np.concatenate([[0], np.cumsum(chunks)]).astype(np.int64)

    with tile.TileContext(nc) as tc:
        with (
            tc.tile_pool(name="const", bufs=1) as cpool,
            tc.tile_pool(name="fpool", bufs=FBLOOK + 1) as fpool,
            tc.tile_pool(name="spool", bufs=6) as spool,
            tc.tile_pool(name="stg", bufs=3) as stpool,
            tc.tile_pool(name="mstg", bufs=2) as mstpool,
            tc.tile_pool(name="mtin", bufs=1) as mtpool,
            tc.tile_pool(name="pscat", bufs=3, space="PSUM") as pspool,
            tc.tile_pool(name="pmap", bufs=1, space="PSUM") as pmpool,
            tc.tile_pool(name="pwarm", bufs=1, space="PSUM") as pwpool,
        ):
            # PE warmup: back-to-back dummy matmuls with no inputs so the
            # HAM clock gate reaches 2.4GHz before real work (without this,
            # matmuls run at the cold 1.2GHz rate).  Re-warm bursts are
            # emitted inside the main loop: one PE stall >3.4us re-throttles
            # the clock and the normal stream is never dense enough to
            # recover on its own.
            wsrc = cpool.tile([128, TILE], f16)
            nc.gpsimd.memset(wsrc[:], 0.0)

            def warm_burst(n):
                wps = pwpool.tile([128, TILE], f32, name="wps")
                for _ in range(n):
                    nc.tensor.matmul(out=wps[:], lhsT=wsrc[:, 0:128],
                                     rhs=wsrc[:], start=True, stop=True)

            warm_burst(NWARM)

            # scatter-critical load first: pos table (alone on its queue)
            posT = cpool.tile([hmax, ncols], f32)
            nc.scalar.dma_start(out=posT[:], in_=post_d[:])

            fbs = {}

            def load_fb(g):
                p0, p1 = g * SG, min((g + 1) * SG, NP)
                c0, c1 = int(colbase[p0]), int(colbase[p1])
                assert c1 - c0 <= FBMAX, (c0, c1)
                fb = fpool.tile([hmax, FBMAX * 2 * C], f16, name="fb")
                nc.sync.dma_start(out=fb[:, :(c1 - c0) * 2 * C],
                                  in_=feat_d[:, c0 * 2 * C:c1 * 2 * C])
                fbs[g] = (fb, c0, c1)

            for g in range(min(FBLOOK, NSG)):
                load_fb(g)

            # constants built on-device (no DMA -> no queue contention)
            iota_i = cpool.tile([128, TILE], mybir.dt.int32)
            nc.gpsimd.iota(iota_i[:], pattern=[[1, TILE]], base=0,
                           channel_multiplier=0)
            iota_f32 = cpool.tile([128, TILE], f32)
            nc.gpsimd.tensor_copy(iota_f32[:], iota_i[:])
            iota_f = cpool.tile([128, TILE], f16)
            nc.vector.tensor_copy(iota_f[:], iota_f32[:])
            ident = cpool.tile([128, 128], f16)
            make_identity(nc, ident[:])

            # map input rides the idle GpSimd (SWDGE) queue, split into
            # y-halves so the first map units unblock early
            mts = []
            for x0, w in XCH:
                mt = mtpool.tile([128, NYH * CM], f16, tag="mt%d" % x0)
                mts.append(mt)
            for xi, (x0, w) in enumerate(XCH):
                nc.gpsimd.dma_start(out=mts[xi][:w, :MHALF],
                                    in_=map_d[x0:x0 + w, :MHALF])
            for xi, (x0, w) in enumerate(XCH):
                nc.gpsimd.dma_start(out=mts[xi][:w, MHALF:],
                                    in_=map_d[x0:x0 + w, MHALF:])

            # ---- map transpose machinery ----
            # unit k2 covers y-blocks 2*k2, 2*k2+1 (last unit: 1 block)
            mstate = {"ms": None}

            def emit_map_unit(k2):
                nb = 2 if 2 * k2 + 1 < NYB else 1
                pm = pmpool.tile([128, 2 * NX], f16, name="pm")
                for j in range(nb):
                    kb = 2 * k2 + j
                    for xi, (x0, w) in enumerate(XCH):
                        nc.tensor.transpose(
                            out=pm[:, j * NX + x0:j * NX + x0 + w],
                            in_=mts[xi][:w, kb * 128:(kb + 1) * 128],
                            identity=ident[:w, :w])
                if k2 % 2 == 0:
                    mstate["ms"] = mstpool.tile([128, MBG * NX], f16,
                                                name="ms")
                ms = mstate["ms"]
                off = (k2 % 2) * 2 * NX
                nc.vector.tensor_copy(out=ms[:, off:off + nb * NX],
                                      in_=pm[:, :nb * NX])
                if k2 % 2 == 1 or k2 == NMU - 1:
                    blk0 = (k2 - k2 % 2) * 2
                    wm = (min(blk0 + MBG, NYB) - blk0) * NX
                    nc.sync.dma_start(
                        out=mapo_d[:, blk0 * NX:blk0 * NX + wm],
                        in_=ms[:, :wm])

            # ---- scatter main loop ----
            emitted_units = 0
            ndp = 0     # global double-pair counter (for copy-engine split)
            for g in range(NSG):
                p0, p1 = g * SG, min((g + 1) * SG, NP)
                fb, c0, c1 = fbs[g]
                stg = stpool.tile([128, SG * TILE], f16, name="stg")
                pr = p0
                while pr < p1:
                    npair = min(2, p1 - pr)
                    ps = pspool.tile([128, 2 * TILE], f32, name="ps")
                    for q in range(npair):
                        pcur = pr + q
                        nck = int(chunks[pcur])
                        for k in range(nck):
                            col = int(colbase[pcur]) + k
                            s_t = spool.tile([hmax, TILE], f16, name="s_t")
                            nc.vector.tensor_scalar(
                                out=s_t[:], in0=iota_f[:hmax],
                                scalar1=posT[:, col:col + 1], scalar2=None,
                                op0=mybir.AluOpType.is_equal)
                            nc.tensor.matmul(
                                out=ps[:, q * TILE:(q + 1) * TILE],
                                lhsT=fb[:, (col - c0) * 2 * C:
                                        (col - c0 + 1) * 2 * C],
                                rhs=s_t[:],
                                start=(k == 0), stop=(k == nck - 1))
                    off = (pr - p0) * TILE
                    # most copies on ACT; every 5th on DVE for balance
                    if ndp % 5 == 4:
                        nc.vector.tensor_copy(
                            out=stg[:, off:off + npair * TILE],
                            in_=ps[:, :npair * TILE])
                    else:
                        nc.scalar.copy(
                            out=stg[:, off:off + npair * TILE],
                            in_=ps[:, :npair * TILE])
                    ndp += 1
                    pr += npair
                a0 = p0 * TILE
                wa = (p1 - p0) * TILE
                nc.sync.dma_start(out=canv_d[:, a0:a0 + wa],
                                  in_=stg[:, :wa])
                if g + FBLOOK < NSG:
                    load_fb(g + FBLOOK)
                if g in REWARM:
                    warm_burst(12)
                if g >= MAP_AT:
                    while (emitted_units < NMU
                           and emitted_units * (MAP_END - MAP_AT)
                           < (g + 1 - MAP_AT) * NMU):
                        emit_map_unit(emitted_units)
                        emitted_units += 1
            while emitted_units < NMU:
                emit_map_unit(emitted_units)
                emitted_units += 1

    nc.finalize()
    return nc


def _host_prep(voxel_features, coords, map_fm):
    """Shard points by core, build fp16 feature/pos tables (index work)."""
    vf = np.asarray(voxel_features)
    cd = np.asarray(coords)
    mf = np.asarray(map_fm)
    if mf.ndim == 5:
        mf = np.squeeze(mf, 3)

    b = cd[:, 0].astype(np.int64)
    y = cd[:, 2].astype(np.int64)
    x = cd[:, 3].astype(np.int64)
    valid = (b >= 0) & (b < B) & (y >= 0) & (y < NY) & (x >= 0) & (x < NX)
    b, y, x = b[valid], y[valid], x[valid]
    vfv = np.ascontiguousarray(vf[valid]).astype(np.float16)

    half = (y >= NYH).astype(np.int64)
    core = b * 2 + half
    lcell = (y - half * NYH) * NX + x
    t = lcell // TILE          # 512-cell tile id
    pos = lcell - t * TILE     # position within tile (= matmul column)
    pair = t % NP              # tile j pairs with tile j+NP
    blk = t // NP              # channel block within the pair

    key = core * NP + pair
    order = np.argsort(key, kind="stable")
    ks = key[order]
    counts = np.bincount(ks, minlength=NCORE * NP)
    kmax = counts.reshape(NCORE, NP).max(axis=0)
    # table height: round max points-per-column up to a multiple of 16
    hmax = int(min(CAP, max(16, -(-int(kmax.max()) // 16) * 16)))
    chunks = np.maximum((kmax + hmax - 1) // hmax, 1)
    for g in range(0, NP, SG):
        need = int(chunks[g:g + SG].sum())
        if need > FBMAX:
            raise ValueError("pair group needs %d cols > FBMAX=%d"
                             % (need, FBMAX))
    ncols = int(chunks.sum())
    colbase = np.concatenate([[0], np.cumsum(chunks)]).astype(np.int64)

    starts = np.concatenate([[0], np.cumsum(counts)]).astype(np.int64)
    rank = np.arange(len(ks), dtype=np.int64) - starts[ks]

    co = core[order]
    po = pair[order]
    bo = blk[order]
    colo = colbase[po] + rank // hmax
    slot = rank % hmax

    feat = np.zeros((NCORE, hmax, ncols, 2 * C), np.float16)
    post = np.full((NCORE, hmax, ncols), -1.0, np.float32)
    ccol = bo[:, None] * C + np.arange(C)[None, :]
    feat[co[:, None], slot[:, None], colo[:, None], ccol] = vfv[order]
    post[co, slot, colo] = pos[order].astype(np.float32)

    maps = []
    for core_id in range(NCORE):
        bb, hh = core_id // 2, core_id % 2
        maps.append(np.ascontiguousarray(
            mf[bb, :, hh * NYH:(hh + 1) * NYH, :]).astype(
                np.float16).reshape(NX, NYH * CM))
    return feat, post, maps, ncols, chunks, hmax


def kernel(voxel_features, coords, batch_size=None, map_fm=None,
           trace=False, _return_results=False):
    from concourse.bass_utils import run_bass_kernel_spmd

    feat, post, maps, ncols, chunks, hmax = _host_prep(
        voxel_features, coords, map_fm)

    ckey = (ncols, hmax, tuple(int(c) for c in chunks))
    if ckey not in _prog_cache:
        _prog_cache.clear()
        _prog_cache[ckey] = _build_program(ncols, chunks, hmax)
    nc = _prog_cache[ckey]

    in_maps = [
        {"feat": feat[i].reshape(hmax, -1), "post": post[i],
         "mapin": maps[i]}
        for i in range(NCORE)
    ]
    res = run_bass_kernel_spmd(nc, in_maps, list(range(NCORE)), trace=trace)

    out = np.empty((B, C + CM, NY, NX), np.float32)
    for core_id in range(NCORE):
        bb, hh = core_id // 2, core_id % 2
        canv = res.results[core_id]["canv"]          # [128, ACELL] f16
        full = np.concatenate(
            [canv[0:C], canv[C:, :BCELL]], axis=1).astype(np.float32)
        out[bb, :C, hh * NYH:(hh + 1) * NYH, :] = full.reshape(C, NYH, NX)
        mo = res.results[core_id]["mapo"]            # [128, NYB*NX] f16
        out[bb, C:, hh * NYH:(hh + 1) * NYH, :] = (
            mo.reshape(YB, CM, NYB, NX).transpose(1, 2, 0, 3)
            .astype(np.float32).reshape(CM, NYH, NX))
    if _return_results:
        return out, res
    return out
